# revision 11
# baseline (speedup 1.0000x reference)
"""BGFusionBlock Trainium2 kernel (Bass/Tile, 8 NeuronCores, SPMD).

Shapes: aligned_feat [4, 8, 64, 128, 128] f32, w1/w2 [64, 64, 3, 3],
b1/b2 [64], wf [64, 512, 1, 1], bf [64].  Output [4, 64, 128, 128] f32.

Math:
  emb     = conv3x3(x, w2, b2)   per frame
  emb_ref = conv3x3(x, w1, b1)   per frame
  scores[b,t,p] = <emb_ref[b,t,:,p], sum_j emb[b,j,:,p]>
  attn = softmax(scores / 0.5, axis=t)
  out  = leaky_relu(conv1x1(aligned_feat * attn, wf, bf), 0.1)

Distribution: shard H across the 8 cores (16 rows each, with a 1-row halo
baked into the per-core input by the host).  All compute is core-local.

Per-core plan (each (b, half-slab of 8 rows) is one unit, px = 8*128 = 1024):
 - Host pre-stages the input as [128p, 8t, 9r, 130c] tiles where partitions
   0:64 hold channel c at slab row r and partitions 64:128 hold channel c at
   slab row r+1 (columns zero-padded).  That makes K=128 matmuls cover two
   vertical conv taps at once.
 - 3x3 convs as float32r matmuls, M=128 packing BOTH convs (w1|w2; swapped
   on odd frames).  The di=2 tail (K=64) is row-tiled: chunk0 runs from
   partitions 0:64 and chunk1 from 64:128 so the PE overlaps the pair.
 - emb summed over t by identity-matmuls into PSUM; scores by ones-matmuls
   over channel products; softmax via PE transposes into px-on-partition
   layout; attn broadcast to (t,c) partitions by 0/1-matmuls; 1x1 conv as a
   K=512 matmul in 4 K-tiles; LeakyReLU on the scalar engine.
"""

import sys
import os
import numpy as np

if "/opt/trn_rl_repo" not in sys.path:
    sys.path.insert(0, "/opt/trn_rl_repo")

B, T, C, H, W = 4, 8, 64, 128, 128
NCORES = 8
RPC = H // NCORES          # rows per core (16)
HS = RPC // 2              # rows per half-slab (8)
PX = HS * W                # pixels per half-slab (1024)
NCH = PX // 512            # 512-wide chunks per half-slab (2)

_CACHE: dict = {}


# ----------------------------------------------------------------------------
# host-side input staging
# ----------------------------------------------------------------------------

def _stage_inputs(aligned_feat):
    """[B,T,C,H,W] -> per-core [B, 2, 128, T, 9, 130] dual-bank padded tiles."""
    af = np.ascontiguousarray(aligned_feat, dtype=np.float32)
    P = np.zeros((B, T, C, H + 2, W), np.float32)
    P[:, :, :, 1:-1, :] = af
    # rows[k, s, r] = 16k + 8s + r  (padded row index of slab row r)
    rows = (16 * np.arange(NCORES)[:, None, None]
            + HS * np.arange(2)[None, :, None]
            + np.arange(9)[None, None, :])
    A = np.zeros((NCORES, B, 2, 128, T, 9, 130), np.float32)
    # lo bank: channel c at slab row r; hi bank: channel c at slab row r+1
    Vlo = P[:, :, :, rows, :]          # [B,T,C,8,2,9,W]
    Vhi = P[:, :, :, rows + 1, :]
    A[:, :, :, :64, :, :, 1:129] = Vlo.transpose(3, 0, 4, 2, 1, 5, 6)
    A[:, :, :, 64:, :, :, 1:129] = Vhi.transpose(3, 0, 4, 2, 1, 5, 6)
    return A


def _make_consts(w1, b1, w2, b2, wf, bf):
    w1 = np.asarray(w1, np.float32); w2 = np.asarray(w2, np.float32)
    b1 = np.asarray(b1, np.float32); b2 = np.asarray(b2, np.float32)
    wf = np.asarray(wf, np.float32).reshape(C, T * C)
    bf = np.asarray(bf, np.float32)

    # conv lhsT: k = di*64 + cc (di in {0,1}); m = conv*64 + oc
    # (conv order swapped on odd frames so pair tiles assemble lane-locked)
    wta = np.zeros((128, 2, 3, 128), np.float32)
    wtb = np.zeros((128, 2, 3, 128), np.float32)
    for sw, (wa, wb) in enumerate([(w1, w2), (w2, w1)]):
        for dj in range(3):
            for di in range(2):
                wta[di * 64:(di + 1) * 64, sw, dj, :64] = wa[:, :, di, dj].T
                wta[di * 64:(di + 1) * 64, sw, dj, 64:] = wb[:, :, di, dj].T
            wtb[:64, sw, dj, :64] = wa[:, :, 2, dj].T
            wtb[:64, sw, dj, 64:] = wb[:, :, 2, dj].T
            wtb[64:, sw, dj, :] = wtb[:64, sw, dj, :]

    wsum = np.zeros((128, 128), np.float32)
    kk = np.arange(128)
    wsum[kk, kk % 64] = 1.0
    wsum[kk, kk % 64 + 64] = 1.0

    wsc = np.zeros((128, 4, 8), np.float32)
    for j in range(4):
        wsc[:64, j, 2 * j] = 1.0
        wsc[64:, j, 2 * j + 1] = 1.0

    we = np.zeros((8, 4, 128), np.float32)
    for j in range(4):
        we[2 * j, j, :64] = 1.0
        we[2 * j + 1, j, 64:] = 1.0

    wwf = np.zeros((128, 4, 128), np.float32)
    for j in range(4):
        for p in range(2):
            blk = wf[:, (2 * j + p) * 64:(2 * j + p + 1) * 64].T  # [cc, oc]
            wwf[p * 64:(p + 1) * 64, j, :64] = blk
            wwf[p * 64:(p + 1) * 64, j, 64:] = blk

    bias = np.zeros((128, 3), np.float32)
    bias[:64, 0] = b1; bias[64:, 0] = b1
    bias[:64, 1] = 8.0 * b2; bias[64:, 1] = 8.0 * b2
    bias[:64, 2] = bf; bias[64:, 2] = bf

    return {
        "wta": wta, "wtb": wtb, "wsum": wsum, "wsc": wsc, "we": we,
        "wwf": wwf, "bias": bias,
        "idt8": np.eye(8, dtype=np.float32),
        "idt128": np.eye(128, dtype=np.float32),
    }


CONST_SHAPES = {
    "wta": (128, 2, 3, 128), "wtb": (128, 2, 3, 128), "wsum": (128, 128),
    "wsc": (128, 4, 8), "we": (8, 4, 128), "wwf": (128, 4, 128),
    "bias": (128, 3), "idt8": (8, 8), "idt128": (128, 128),
}


# ----------------------------------------------------------------------------
# kernel program
# ----------------------------------------------------------------------------

def build_nc():
    import concourse.bass as bass
    import concourse.tile as tile
    import concourse.mybir as mybir
    from concourse import bacc
    from contextlib import ExitStack

    f32 = mybir.dt.float32
    f32r = mybir.dt.float32r
    ACT = mybir.ActivationFunctionType
    ALU = mybir.AluOpType
    AX = mybir.AxisListType

    nc = bacc.Bacc("TRN2", target_bir_lowering=False, debug=False)

    MM_CONSTS = {"wta", "wtb", "wsum", "wsc", "we", "wwf"}
    x = nc.dram_tensor("x", [B, 2, 128, T, 9, 130], f32r,
                       kind="ExternalInput").ap()
    cst = {n: nc.dram_tensor(n, list(s), f32r if n in MM_CONSTS else f32,
                             kind="ExternalInput").ap()
           for n, s in CONST_SHAPES.items()}
    out = nc.dram_tensor("out", [B, 128, HS, W], f32, kind="ExternalOutput").ap()

    def r(ap):
        return ap.bitcast(f32r)

    with tile.TileContext(nc) as tc, ExitStack() as ctx:
        cp = ctx.enter_context(tc.tile_pool(name="const", bufs=1))
        consts = {}
        for n, s in CONST_SHAPES.items():
            t = cp.tile(list(s), f32r if n in MM_CONSTS else f32, tag=n)
            nc.sync.dma_start(t[:], cst[n][:])
            consts[n] = t
        wta, wtb = consts["wta"], consts["wtb"]
        wsum, wsc, we, wwf = (consts["wsum"], consts["wsc"], consts["we"],
                              consts["wwf"])
        bias, idt8, idt128 = consts["bias"], consts["idt8"], consts["idt128"]

        xx_pool = ctx.enter_context(tc.tile_pool(name="xx", bufs=2))
        ref_pool = ctx.enter_context(tc.tile_pool(name="ref", bufs=5))
        emb_pool = ctx.enter_context(tc.tile_pool(name="emb", bufs=2))
        sml_pool = ctx.enter_context(tc.tile_pool(name="sml", bufs=2))
        prod_pool = ctx.enter_context(tc.tile_pool(name="prodp", bufs=5))
        out_pool = ctx.enter_context(tc.tile_pool(name="outb", bufs=2))
        ps1 = ctx.enter_context(tc.tile_pool(name="ps1", bufs=3, space="PSUM"))
        pss = ctx.enter_context(tc.tile_pool(name="pss", bufs=2, space="PSUM"))
        ps3 = ctx.enter_context(tc.tile_pool(name="ps3", bufs=3, space="PSUM"))

        state = {}

        def phase_a(u):
            b, s = divmod(u, 2)
            xx = xx_pool.tile([128, T, 9, 130], f32r, tag="xx",
                              name=f"xx{u}")
            nc.sync.dma_start(xx[:], x[b, s])

            sums = [pss.tile([128, 512], f32, tag="sum", name=f"sum{u}_{ch}")
                    for ch in range(NCH)]
            refs = []
            for j in range(4):
                refj = ref_pool.tile([128, NCH, 512], f32, tag="embref",
                                     name=f"ref{u}_{j}")
                embj = emb_pool.tile([128, NCH, 512], f32r, tag="emb",
                                     name=f"emb{u}_{j}")
                for p in range(2):
                    t = 2 * j + p
                    sw = t % 2
                    cps = [ps1.tile([128, 512], f32, tag="cps",
                                    name=f"cps{u}_{t}_{ch}")
                           for ch in range(NCH)]
                    for ch in range(NCH):
                        r0 = 4 * ch
                        for dj in range(3):
                            nc.tensor.matmul(
                                cps[ch][:], wta[:, sw, dj, :],
                                xx[:, t, r0:r0 + 4, dj:dj + 128],
                                start=(dj == 0), stop=False)
                    for dj in range(3):
                        nc.tensor.matmul(
                            cps[0][:], wtb[0:64, sw, dj, :],
                            xx[0:64, t, 2:6, dj:dj + 128],
                            start=False, stop=(dj == 2))
                        nc.tensor.matmul(
                            cps[1][:], wtb[64:128, sw, dj, :],
                            xx[64:128, t, 5:9, dj:dj + 128],
                            start=False, stop=(dj == 2))
                    for ch in range(NCH):
                        if sw == 0:
                            nc.scalar.activation(
                                refj[0:64, ch, :], cps[ch][0:64, :],
                                ACT.Identity, bias=bias[0:64, 0:1])
                            nc.scalar.activation(
                                embj[64:128, ch, :], cps[ch][64:128, :],
                                ACT.Copy)
                        else:
                            nc.scalar.activation(
                                embj[0:64, ch, :], cps[ch][0:64, :],
                                ACT.Copy)
                            nc.scalar.activation(
                                refj[64:128, ch, :], cps[ch][64:128, :],
                                ACT.Identity, bias=bias[64:128, 0:1])
                for ch in range(NCH):
                    nc.tensor.matmul(sums[ch][:], wsum[:], embj[:, ch, :],
                                     start=(j == 0), stop=(j == 3))
                refs.append(refj)

            esum = sml_pool.tile([128, NCH, 512], f32, tag="esum",
                                 name=f"esum{u}")
            for ch in range(NCH):
                nc.scalar.activation(esum[:, ch, :], sums[ch][:],
                                     ACT.Identity, bias=bias[:, 1:2])

            scps = [ps3.tile([8, 512], f32, tag="soft",
                             name=f"scps{u}_{ch}") for ch in range(NCH)]
            prods = []
            for j in range(4):
                prodj = prod_pool.tile([128, NCH, 512], f32r, tag="prod",
                                       name=f"prod{u}_{j}")
                nc.vector.tensor_mul(prodj[:].opt(), refs[j][:].opt(),
                                     esum[:].opt())
                prods.append(prodj)
            for j in range(4):
                for ch in range(NCH):
                    nc.tensor.matmul(scps[ch][:], wsc[:, j, :],
                                     prods[j][:, ch, :],
                                     start=(j == 0), stop=(j == 3))
            state[u] = (xx, scps)

        def phase_b(u):
            b, s = divmod(u, 2)
            xx, scps = state.pop(u)
            scs = sml_pool.tile([8, NCH, 512], f32, tag="scs",
                                name=f"scs{u}")
            for ch in range(NCH):
                nc.scalar.activation(scs[:, ch, :], scps[ch][:],
                                     ACT.Identity, scale=2.0)
            psT = pss.tile([128, HS, 8], f32, tag="sum", name=f"psT{u}")
            for h in range(HS):
                nc.tensor.transpose(psT[:, h, :],
                                    scs[:].opt()[:, 128 * h:128 * (h + 1)],
                                    idt8[:])
            nm = sml_pool.tile([128, HS], f32, tag="nm", name=f"nm{u}")
            nc.vector.tensor_reduce(nm[:], psT[:], axis=AX.X, op=ALU.max,
                                    negate=True)
            et = sml_pool.tile([128, HS, 8], f32, tag="et", name=f"et{u}")
            for h in range(HS):
                nc.scalar.activation(et[:, h, :], psT[:, h, :], ACT.Exp,
                                     bias=nm[:, h:h + 1])
            den = sml_pool.tile([128, HS], f32, tag="den", name=f"den{u}")
            nc.vector.tensor_reduce(den[:], et[:], axis=AX.X, op=ALU.add)
            rec = sml_pool.tile([128, HS], f32, tag="rec", name=f"rec{u}")
            nc.vector.reciprocal(rec[:], den[:])
            attnT = sml_pool.tile([128, HS, 8], f32, tag="attnT",
                                  name=f"attnT{u}")
            for h in range(HS):
                nc.vector.tensor_scalar_mul(attnT[:, h, :], et[:, h, :],
                                            rec[:, h:h + 1])
            attn_ps = [ps3.tile([8, 512], f32, tag="soft",
                                name=f"attnps{u}_{ch}") for ch in range(NCH)]
            for h in range(HS):
                nc.tensor.transpose(
                    attn_ps[h // 4][:, 128 * (h % 4):128 * (h % 4 + 1)],
                    attnT[:, h, :], idt128[:])
            attn = sml_pool.tile([8, NCH, 512], f32r, tag="attn",
                                 name=f"attn{u}")
            for ch in range(NCH):
                nc.scalar.activation(attn[:, ch, :], attn_ps[ch][:],
                                     ACT.Copy)

            outbuf = state.get(("ob", b))
            if outbuf is None:
                outbuf = out_pool.tile([128, HS, W], f32, tag="outbuf",
                                       name=f"outbuf{b}")
                state[("ob", b)] = outbuf
            ops = [ps1.tile([128, 512], f32, tag="cps",
                            name=f"ops{u}_{ch}") for ch in range(NCH)]
            for j in range(4):
                for ch in range(NCH):
                    abc = ps3.tile([128, 4, 128], f32, tag="soft",
                                   name=f"abc{u}_{j}_{ch}")
                    nc.tensor.matmul(abc[:].opt(), we[:, j, :],
                                     attn[:, ch, :],
                                     start=True, stop=True)
                    wt = sml_pool.tile([128, 4, 128], f32r, tag="wt",
                                       name=f"wt{u}_{j}_{ch}")
                    nc.vector.tensor_mul(
                        wt[0:64],
                        xx[0:64, 2 * j, 1 + 4 * ch:5 + 4 * ch, 1:129]
                        .bitcast(f32),
                        abc[0:64])
                    nc.vector.tensor_mul(
                        wt[64:128],
                        xx[64:128, 2 * j + 1, 4 * ch:4 + 4 * ch, 1:129]
                        .bitcast(f32),
                        abc[64:128])
                    nc.tensor.matmul(ops[ch][:], wwf[:, j, :], wt[:].opt(),
                                     start=(j == 0), stop=(j == 3))
            for ch in range(NCH):
                yb = sml_pool.tile([128, 512], f32, tag="yb",
                                   name=f"yb{u}_{ch}")
                nc.scalar.activation(
                    yb[64 * s:64 * (s + 1), :],
                    ops[ch][64 * s:64 * (s + 1), :],
                    ACT.Identity, bias=bias[64 * s:64 * (s + 1), 2:3])
                nc.vector.scalar_tensor_tensor(
                    outbuf[64 * s:64 * (s + 1),
                           4 * ch:4 * (ch + 1), :].opt(),
                    yb[64 * s:64 * (s + 1), :], 0.1,
                    yb[64 * s:64 * (s + 1), :],
                    op0=ALU.mult, op1=ALU.max)
            if s == 1:
                nc.sync.dma_start(out[b], outbuf[:])
                del state[("ob", b)]

        phase_a(0)
        for u in range(1, 8):
            phase_a(u)
            phase_b(u - 1)
        phase_b(7)

    nc.compile()
    return nc


# ----------------------------------------------------------------------------
# entry point
# ----------------------------------------------------------------------------

def kernel(aligned_feat, w1, b1, w2, b2, wf, bf):
    from concourse import bass_utils

    if "nc" not in _CACHE:
        _CACHE["nc"] = build_nc()
    nc = _CACHE["nc"]

    A = _stage_inputs(aligned_feat)
    consts = _make_consts(w1, b1, w2, b2, wf, bf)
    in_maps = [{"x": A[k], **consts} for k in range(NCORES)]

    res = bass_utils.run_bass_kernel_spmd(nc, in_maps, core_ids=list(range(NCORES)))
    outs = [res.results[k]["out"] for k in range(NCORES)]  # [B,128,HS,W]

    full = np.empty((B, C, H, W), np.float32)
    for k in range(NCORES):
        o = outs[k]
        for s in range(2):
            full[:, :, RPC * k + HS * s: RPC * k + HS * (s + 1), :] = \
                o[:, 64 * s:64 * (s + 1), :, :]
    return full


# revision 12
# speedup vs baseline: 1.1793x; 1.1793x over previous
"""BGFusionBlock Trainium2 kernel (Bass/Tile, 8 NeuronCores, SPMD).

Shapes: aligned_feat [4, 8, 64, 128, 128] f32, w1/w2 [64, 64, 3, 3],
b1/b2 [64], wf [64, 512, 1, 1], bf [64].  Output [4, 64, 128, 128] f32.

Math:
  emb     = conv3x3(x, w2, b2)   per frame
  emb_ref = conv3x3(x, w1, b1)   per frame
  scores[b,t,p] = <emb_ref[b,t,:,p], sum_j emb[b,j,:,p]>
  attn = softmax(scores / 0.5, axis=t)
  out  = leaky_relu(conv1x1(aligned_feat * attn, wf, bf), 0.1)

Distribution: shard H across the 8 cores (16 rows each, with a 1-row halo
baked into the per-core input by the host).  All compute is core-local.

Per-core plan (each (b, half-slab of 8 rows) is one unit, px = 8*128 = 1024):
 - Host pre-stages the input as [128p, 8t, 9r, 130c] tiles where partitions
   0:64 hold channel c at slab row r and partitions 64:128 hold channel c at
   slab row r+1 (columns zero-padded).  That makes K=128 matmuls cover two
   vertical conv taps at once.
 - 3x3 convs as float32r matmuls, M=128 packing BOTH convs (w1|w2; swapped
   on odd frames).  The di=2 tail (K=64) is row-tiled: chunk0 runs from
   partitions 0:64 and chunk1 from 64:128 so the PE overlaps the pair.
 - emb summed over t by identity-matmuls into PSUM; scores by ones-matmuls
   over channel products; softmax via PE transposes into px-on-partition
   layout; attn broadcast to (t,c) partitions by 0/1-matmuls; 1x1 conv as a
   K=512 matmul in 4 K-tiles; LeakyReLU on the scalar engine.
"""

import sys
import os
import numpy as np

if "/opt/trn_rl_repo" not in sys.path:
    sys.path.insert(0, "/opt/trn_rl_repo")

B, T, C, H, W = 4, 8, 64, 128, 128
NCORES = 8
RPC = H // NCORES          # rows per core (16)
HS = RPC // 2              # rows per half-slab (8)
PX = HS * W                # pixels per half-slab (1024)
NCH = PX // 512            # 512-wide chunks per half-slab (2)

_CACHE: dict = {}


# ----------------------------------------------------------------------------
# host-side input staging
# ----------------------------------------------------------------------------

def _stage_inputs(aligned_feat):
    """[B,T,C,H,W] -> per-core [B, 2, 128, T, 9, 130] dual-bank padded tiles."""
    af = np.ascontiguousarray(aligned_feat, dtype=np.float32)
    P = np.zeros((B, T, C, H + 2, W), np.float32)
    P[:, :, :, 1:-1, :] = af
    # rows[k, s, r] = 16k + 8s + r  (padded row index of slab row r)
    rows = (16 * np.arange(NCORES)[:, None, None]
            + HS * np.arange(2)[None, :, None]
            + np.arange(9)[None, None, :])
    A = np.zeros((NCORES, B, 2, 128, T, 9, 130), np.float32)
    # lo bank: channel c at slab row r; hi bank: channel c at slab row r+1
    Vlo = P[:, :, :, rows, :]          # [B,T,C,8,2,9,W]
    Vhi = P[:, :, :, rows + 1, :]
    A[:, :, :, :64, :, :, 1:129] = Vlo.transpose(3, 0, 4, 2, 1, 5, 6)
    A[:, :, :, 64:, :, :, 1:129] = Vhi.transpose(3, 0, 4, 2, 1, 5, 6)
    return A


def _make_consts(w1, b1, w2, b2, wf, bf):
    w1 = np.asarray(w1, np.float32); w2 = np.asarray(w2, np.float32)
    b1 = np.asarray(b1, np.float32); b2 = np.asarray(b2, np.float32)
    wf = np.asarray(wf, np.float32).reshape(C, T * C)
    bf = np.asarray(bf, np.float32)

    # conv lhsT: k = di*64 + cc (di in {0,1}); m = conv*64 + oc
    # (conv order swapped on odd frames so pair tiles assemble lane-locked)
    wta = np.zeros((128, 2, 3, 128), np.float32)
    wtb = np.zeros((128, 2, 3, 128), np.float32)
    for sw, (wa, wb) in enumerate([(w1, w2), (w2, w1)]):
        for dj in range(3):
            for di in range(2):
                wta[di * 64:(di + 1) * 64, sw, dj, :64] = wa[:, :, di, dj].T
                wta[di * 64:(di + 1) * 64, sw, dj, 64:] = wb[:, :, di, dj].T
            wtb[:64, sw, dj, :64] = wa[:, :, 2, dj].T
            wtb[:64, sw, dj, 64:] = wb[:, :, 2, dj].T
            wtb[64:, sw, dj, :] = wtb[:64, sw, dj, :]

    wsum = np.zeros((128, 128), np.float32)
    kk = np.arange(128)
    wsum[kk, kk % 64] = 1.0
    wsum[kk, kk % 64 + 64] = 1.0

    wsc = np.zeros((128, 4, 8), np.float32)
    for j in range(4):
        wsc[:64, j, 2 * j] = 1.0
        wsc[64:, j, 2 * j + 1] = 1.0

    we = np.zeros((8, 4, 128), np.float32)
    for j in range(4):
        we[2 * j, j, :64] = 1.0
        we[2 * j + 1, j, 64:] = 1.0

    wwf = np.zeros((128, 4, 128), np.float32)
    for j in range(4):
        for p in range(2):
            blk = wf[:, (2 * j + p) * 64:(2 * j + p + 1) * 64].T  # [cc, oc]
            wwf[p * 64:(p + 1) * 64, j, :64] = blk
            wwf[p * 64:(p + 1) * 64, j, 64:] = blk

    bias = np.zeros((128, 3), np.float32)
    bias[:64, 0] = b1; bias[64:, 0] = b1
    bias[:64, 1] = 8.0 * b2; bias[64:, 1] = 8.0 * b2
    bias[:64, 2] = bf; bias[64:, 2] = bf

    return {
        "wta": wta, "wtb": wtb, "wsum": wsum, "wsc": wsc, "we": we,
        "wwf": wwf, "bias": bias,
        "idt8": np.eye(8, dtype=np.float32),
        "idt128": np.eye(128, dtype=np.float32),
    }


CONST_SHAPES = {
    "wta": (128, 2, 3, 128), "wtb": (128, 2, 3, 128), "wsum": (128, 128),
    "wsc": (128, 4, 8), "we": (8, 4, 128), "wwf": (128, 4, 128),
    "bias": (128, 3), "idt8": (8, 8), "idt128": (128, 128),
}


# ----------------------------------------------------------------------------
# kernel program
# ----------------------------------------------------------------------------

def build_nc():
    import concourse.bass as bass
    import concourse.tile as tile
    import concourse.mybir as mybir
    from concourse import bacc
    from contextlib import ExitStack

    f32 = mybir.dt.float32
    f32r = mybir.dt.float32r
    ACT = mybir.ActivationFunctionType
    ALU = mybir.AluOpType
    AX = mybir.AxisListType

    nc = bacc.Bacc("TRN2", target_bir_lowering=False, debug=False)

    MM_CONSTS = {"wta", "wtb", "wsum", "wsc", "we", "wwf"}
    x = nc.dram_tensor("x", [B, 2, 128, T, 9, 130], f32r,
                       kind="ExternalInput").ap()
    cst = {n: nc.dram_tensor(n, list(s), f32r if n in MM_CONSTS else f32,
                             kind="ExternalInput").ap()
           for n, s in CONST_SHAPES.items()}
    out = nc.dram_tensor("out", [B, 128, HS, W], f32, kind="ExternalOutput").ap()

    def r(ap):
        return ap.bitcast(f32r)

    with tile.TileContext(nc) as tc, ExitStack() as ctx:
        cp = ctx.enter_context(tc.tile_pool(name="const", bufs=1))
        consts = {}
        for n, s in CONST_SHAPES.items():
            t = cp.tile(list(s), f32r if n in MM_CONSTS else f32, tag=n)
            nc.sync.dma_start(t[:], cst[n][:])
            consts[n] = t
        wta, wtb = consts["wta"], consts["wtb"]
        wsum, wsc, we, wwf = (consts["wsum"], consts["wsc"], consts["we"],
                              consts["wwf"])
        bias, idt8, idt128 = consts["bias"], consts["idt8"], consts["idt128"]

        xx_pool = ctx.enter_context(tc.tile_pool(name="xx", bufs=5))
        ref_pool = ctx.enter_context(tc.tile_pool(name="ref", bufs=4))
        emb_pool = ctx.enter_context(tc.tile_pool(name="emb", bufs=2))
        sml_pool = ctx.enter_context(tc.tile_pool(name="sml", bufs=2))
        prod_pool = ctx.enter_context(tc.tile_pool(name="prodp", bufs=4))
        out_pool = ctx.enter_context(tc.tile_pool(name="outb", bufs=2))
        ps1 = ctx.enter_context(tc.tile_pool(name="ps1", bufs=3, space="PSUM"))
        pss = ctx.enter_context(tc.tile_pool(name="pss", bufs=2, space="PSUM"))
        ps3 = ctx.enter_context(tc.tile_pool(name="ps3", bufs=3, space="PSUM"))

        state = {}

        def phase_a(u):
            b, s = divmod(u, 2)
            xxh = []
            for hf in range(2):
                xt = xx_pool.tile([128, 4, 9, 130], f32r, tag="xx",
                                  name=f"xx{u}_{hf}")
                nc.sync.dma_start(xt[:], x[b, s, :, 4 * hf:4 * (hf + 1)])
                xxh.append(xt)

            def xx(t):
                return xxh[t // 4][:, t % 4]

            sums = [pss.tile([128, 512], f32, tag="sum", name=f"sum{u}_{ch}")
                    for ch in range(NCH)]
            refs = []
            for j in range(4):
                refj = ref_pool.tile([128, NCH, 512], f32, tag="embref",
                                     name=f"ref{u}_{j}")
                embj = emb_pool.tile([128, NCH, 512], f32r, tag="emb",
                                     name=f"emb{u}_{j}")
                for p in range(2):
                    t = 2 * j + p
                    sw = t % 2
                    cps = [ps1.tile([128, 512], f32, tag="cps",
                                    name=f"cps{u}_{t}_{ch}")
                           for ch in range(NCH)]
                    for ch in range(NCH):
                        r0 = 4 * ch
                        for dj in range(3):
                            nc.tensor.matmul(
                                cps[ch][:], wta[:, sw, dj, :],
                                xx(t)[:, r0:r0 + 4, dj:dj + 128],
                                start=(dj == 0), stop=False)
                    for dj in range(3):
                        nc.tensor.matmul(
                            cps[0][:], wtb[0:64, sw, dj, :],
                            xx(t)[0:64, 2:6, dj:dj + 128],
                            start=False, stop=(dj == 2))
                        nc.tensor.matmul(
                            cps[1][:], wtb[64:128, sw, dj, :],
                            xx(t)[64:128, 5:9, dj:dj + 128],
                            start=False, stop=(dj == 2))
                    for ch in range(NCH):
                        if sw == 0:
                            nc.scalar.activation(
                                refj[0:64, ch, :], cps[ch][0:64, :],
                                ACT.Identity, bias=bias[0:64, 0:1])
                            nc.scalar.activation(
                                embj[64:128, ch, :], cps[ch][64:128, :],
                                ACT.Copy)
                        else:
                            nc.scalar.activation(
                                embj[0:64, ch, :], cps[ch][0:64, :],
                                ACT.Copy)
                            nc.scalar.activation(
                                refj[64:128, ch, :], cps[ch][64:128, :],
                                ACT.Identity, bias=bias[64:128, 0:1])
                for ch in range(NCH):
                    nc.tensor.matmul(sums[ch][:], wsum[:], embj[:, ch, :],
                                     start=(j == 0), stop=(j == 3))
                refs.append(refj)

            esum = sml_pool.tile([128, NCH, 512], f32, tag="esum",
                                 name=f"esum{u}")
            for ch in range(NCH):
                nc.scalar.activation(esum[:, ch, :], sums[ch][:],
                                     ACT.Identity, bias=bias[:, 1:2])

            scps = [ps3.tile([8, 512], f32, tag="soft",
                             name=f"scps{u}_{ch}") for ch in range(NCH)]
            prods = []
            for j in range(4):
                prodj = prod_pool.tile([128, NCH, 512], f32r, tag="prod",
                                       name=f"prod{u}_{j}")
                nc.vector.tensor_mul(prodj[:].opt(), refs[j][:].opt(),
                                     esum[:].opt())
                prods.append(prodj)
            for j in range(4):
                for ch in range(NCH):
                    nc.tensor.matmul(scps[ch][:], wsc[:, j, :],
                                     prods[j][:, ch, :],
                                     start=(j == 0), stop=(j == 3))
            state[u] = (xxh, scps)

        def phase_b(u):
            b, s = divmod(u, 2)
            xxh, scps = state.pop(u)

            def xx(t):
                return xxh[t // 4][:, t % 4]

            scs = sml_pool.tile([8, NCH, 512], f32, tag="scs",
                                name=f"scs{u}")
            for ch in range(NCH):
                nc.scalar.activation(scs[:, ch, :], scps[ch][:],
                                     ACT.Identity, scale=2.0)
            psT = pss.tile([128, HS, 8], f32, tag="sum", name=f"psT{u}")
            for h in range(HS):
                nc.tensor.transpose(psT[:, h, :],
                                    scs[:].opt()[:, 128 * h:128 * (h + 1)],
                                    idt8[:])
            nm = sml_pool.tile([128, HS], f32, tag="nm", name=f"nm{u}")
            nc.vector.tensor_reduce(nm[:], psT[:], axis=AX.X, op=ALU.max,
                                    negate=True)
            et = sml_pool.tile([128, HS, 8], f32, tag="et", name=f"et{u}")
            for h in range(HS):
                nc.scalar.activation(et[:, h, :], psT[:, h, :], ACT.Exp,
                                     bias=nm[:, h:h + 1])
            den = sml_pool.tile([128, HS], f32, tag="den", name=f"den{u}")
            nc.vector.tensor_reduce(den[:], et[:], axis=AX.X, op=ALU.add)
            rec = sml_pool.tile([128, HS], f32, tag="rec", name=f"rec{u}")
            nc.vector.reciprocal(rec[:], den[:])
            attnT = sml_pool.tile([128, HS, 8], f32, tag="attnT",
                                  name=f"attnT{u}")
            for h in range(HS):
                nc.vector.tensor_scalar_mul(attnT[:, h, :], et[:, h, :],
                                            rec[:, h:h + 1])
            attn_ps = [ps3.tile([8, 512], f32, tag="soft",
                                name=f"attnps{u}_{ch}") for ch in range(NCH)]
            for h in range(HS):
                nc.tensor.transpose(
                    attn_ps[h // 4][:, 128 * (h % 4):128 * (h % 4 + 1)],
                    attnT[:, h, :], idt128[:])
            attn = sml_pool.tile([8, NCH, 512], f32r, tag="attn",
                                 name=f"attn{u}")
            for ch in range(NCH):
                nc.scalar.activation(attn[:, ch, :], attn_ps[ch][:],
                                     ACT.Copy)

            outbuf = state.get(("ob", b))
            if outbuf is None:
                outbuf = out_pool.tile([128, HS, W], f32, tag="outbuf",
                                       name=f"outbuf{b}")
                state[("ob", b)] = outbuf
            ops = [ps1.tile([128, 512], f32, tag="cps",
                            name=f"ops{u}_{ch}") for ch in range(NCH)]
            for j in range(4):
                for ch in range(NCH):
                    abc = ps3.tile([128, 4, 128], f32, tag="soft",
                                   name=f"abc{u}_{j}_{ch}")
                    nc.tensor.matmul(abc[:].opt(), we[:, j, :],
                                     attn[:, ch, :],
                                     start=True, stop=True)
                    wt = sml_pool.tile([128, 4, 128], f32r, tag="wt",
                                       name=f"wt{u}_{j}_{ch}")
                    nc.vector.tensor_mul(
                        wt[0:64],
                        xx(2 * j)[0:64, 1 + 4 * ch:5 + 4 * ch, 1:129]
                        .bitcast(f32),
                        abc[0:64])
                    nc.vector.tensor_mul(
                        wt[64:128],
                        xx(2 * j + 1)[64:128, 4 * ch:4 + 4 * ch, 1:129]
                        .bitcast(f32),
                        abc[64:128])
                    nc.tensor.matmul(ops[ch][:], wwf[:, j, :], wt[:].opt(),
                                     start=(j == 0), stop=(j == 3))
            for ch in range(NCH):
                yb = sml_pool.tile([128, 512], f32, tag="yb",
                                   name=f"yb{u}_{ch}")
                nc.scalar.activation(
                    yb[64 * s:64 * (s + 1), :],
                    ops[ch][64 * s:64 * (s + 1), :],
                    ACT.Identity, bias=bias[64 * s:64 * (s + 1), 2:3])
                nc.vector.scalar_tensor_tensor(
                    outbuf[64 * s:64 * (s + 1),
                           4 * ch:4 * (ch + 1), :].opt(),
                    yb[64 * s:64 * (s + 1), :], 0.1,
                    yb[64 * s:64 * (s + 1), :],
                    op0=ALU.mult, op1=ALU.max)
            if s == 1:
                nc.sync.dma_start(out[b], outbuf[:])
                del state[("ob", b)]

        phase_a(0)
        for u in range(1, 8):
            phase_a(u)
            phase_b(u - 1)
        phase_b(7)

    nc.compile()
    return nc


# ----------------------------------------------------------------------------
# entry point
# ----------------------------------------------------------------------------

def kernel(aligned_feat, w1, b1, w2, b2, wf, bf):
    from concourse import bass_utils

    if "nc" not in _CACHE:
        _CACHE["nc"] = build_nc()
    nc = _CACHE["nc"]

    A = _stage_inputs(aligned_feat)
    consts = _make_consts(w1, b1, w2, b2, wf, bf)
    in_maps = [{"x": A[k], **consts} for k in range(NCORES)]

    res = bass_utils.run_bass_kernel_spmd(nc, in_maps, core_ids=list(range(NCORES)))
    outs = [res.results[k]["out"] for k in range(NCORES)]  # [B,128,HS,W]

    full = np.empty((B, C, H, W), np.float32)
    for k in range(NCORES):
        o = outs[k]
        for s in range(2):
            full[:, :, RPC * k + HS * s: RPC * k + HS * (s + 1), :] = \
                o[:, 64 * s:64 * (s + 1), :, :]
    return full


# revision 13
# speedup vs baseline: 1.4237x; 1.2073x over previous
"""BGFusionBlock Trainium2 kernel (Bass/Tile, 8 NeuronCores, SPMD).

Shapes: aligned_feat [4, 8, 64, 128, 128] f32, w1/w2 [64, 64, 3, 3],
b1/b2 [64], wf [64, 512, 1, 1], bf [64].  Output [4, 64, 128, 128] f32.

Math:
  emb     = conv3x3(x, w2, b2)   per frame
  emb_ref = conv3x3(x, w1, b1)   per frame
  scores[b,t,p] = <emb_ref[b,t,:,p], sum_j emb[b,j,:,p]>
  attn = softmax(scores / 0.5, axis=t)
  out  = leaky_relu(conv1x1(aligned_feat * attn, wf, bf), 0.1)

Distribution: shard H across the 8 cores (16 rows each, with a 1-row halo
baked into the per-core input by the host).  All compute is core-local.

Per-core plan (each (b, half-slab of 8 rows) is one unit, px = 8*128 = 1024):
 - Host pre-stages the input as [128p, 8t, 9r, 130c] tiles where partitions
   0:64 hold channel c at slab row r and partitions 64:128 hold channel c at
   slab row r+1 (columns zero-padded).  That makes K=128 matmuls cover two
   vertical conv taps at once.
 - 3x3 convs as float32r matmuls, M=128 packing BOTH convs (w1|w2; swapped
   on odd frames).  The di=2 tail (K=64) is row-tiled: chunk0 runs from
   partitions 0:64 and chunk1 from 64:128 so the PE overlaps the pair.
 - emb summed over t by identity-matmuls into PSUM; scores by ones-matmuls
   over channel products; softmax via PE transposes into px-on-partition
   layout; attn broadcast to (t,c) partitions by 0/1-matmuls; 1x1 conv as a
   K=512 matmul in 4 K-tiles; LeakyReLU on the scalar engine.
"""

import sys
import os
import numpy as np

if "/opt/trn_rl_repo" not in sys.path:
    sys.path.insert(0, "/opt/trn_rl_repo")

B, T, C, H, W = 4, 8, 64, 128, 128
NCORES = 8
RPC = H // NCORES          # rows per core (16)
HS = RPC // 2              # rows per half-slab (8)
PX = HS * W                # pixels per half-slab (1024)
NCH = PX // 512            # 512-wide chunks per half-slab (2)

_CACHE: dict = {}


# ----------------------------------------------------------------------------
# host-side input staging
# ----------------------------------------------------------------------------

def _stage_inputs(aligned_feat):
    """[B,T,C,H,W] -> per-core [B, 2, 128, T, 9, 130] dual-bank padded tiles."""
    af = np.ascontiguousarray(aligned_feat, dtype=np.float32)
    P = np.zeros((B, T, C, H + 2, W), np.float32)
    P[:, :, :, 1:-1, :] = af
    # rows[k, s, r] = 16k + 8s + r  (padded row index of slab row r)
    rows = (16 * np.arange(NCORES)[:, None, None]
            + HS * np.arange(2)[None, :, None]
            + np.arange(9)[None, None, :])
    A = np.zeros((NCORES, B, 2, 128, T, 9, 130), np.float32)
    # lo bank: channel c at slab row r; hi bank: channel c at slab row r+1
    Vlo = P[:, :, :, rows, :]          # [B,T,C,8,2,9,W]
    Vhi = P[:, :, :, rows + 1, :]
    A[:, :, :, :64, :, :, 1:129] = Vlo.transpose(3, 0, 4, 2, 1, 5, 6)
    A[:, :, :, 64:, :, :, 1:129] = Vhi.transpose(3, 0, 4, 2, 1, 5, 6)
    return A


def _make_consts(w1, b1, w2, b2, wf, bf):
    w1 = np.asarray(w1, np.float32); w2 = np.asarray(w2, np.float32)
    b1 = np.asarray(b1, np.float32); b2 = np.asarray(b2, np.float32)
    wf = np.asarray(wf, np.float32).reshape(C, T * C)
    bf = np.asarray(bf, np.float32)

    # conv lhsT: k = di*64 + cc (di in {0,1}); m = conv*64 + oc
    # (conv order swapped on odd frames so pair tiles assemble lane-locked)
    wta = np.zeros((128, 2, 3, 128), np.float32)
    wtb = np.zeros((128, 2, 3, 128), np.float32)
    for sw, (wa, wb) in enumerate([(w1, w2), (w2, w1)]):
        for dj in range(3):
            for di in range(2):
                wta[di * 64:(di + 1) * 64, sw, dj, :64] = wa[:, :, di, dj].T
                wta[di * 64:(di + 1) * 64, sw, dj, 64:] = wb[:, :, di, dj].T
            wtb[:64, sw, dj, :64] = wa[:, :, 2, dj].T
            wtb[:64, sw, dj, 64:] = wb[:, :, 2, dj].T
            wtb[64:, sw, dj, :] = wtb[:64, sw, dj, :]

    wsum = np.zeros((128, 128), np.float32)
    kk = np.arange(128)
    wsum[kk, kk % 64] = 1.0
    wsum[kk, kk % 64 + 64] = 1.0

    wsc = np.zeros((128, 4, 8), np.float32)
    for j in range(4):
        wsc[:64, j, 2 * j] = 1.0
        wsc[64:, j, 2 * j + 1] = 1.0

    we = np.zeros((8, 4, 128), np.float32)
    for j in range(4):
        we[2 * j, j, :64] = 1.0
        we[2 * j + 1, j, 64:] = 1.0

    wwf = np.zeros((128, 4, 128), np.float32)
    for j in range(4):
        for p in range(2):
            blk = wf[:, (2 * j + p) * 64:(2 * j + p + 1) * 64].T  # [cc, oc]
            wwf[p * 64:(p + 1) * 64, j, :64] = blk
            wwf[p * 64:(p + 1) * 64, j, 64:] = blk

    bias = np.zeros((128, 3), np.float32)
    bias[:64, 0] = b1; bias[64:, 0] = b1
    bias[:64, 1] = 8.0 * b2; bias[64:, 1] = 8.0 * b2
    bias[:64, 2] = bf; bias[64:, 2] = bf

    return {
        "wta": wta, "wtb": wtb, "wsum": wsum, "wsc": wsc, "we": we,
        "wwf": wwf, "bias": bias,
        "idt8": np.eye(8, dtype=np.float32),
        "idt128": np.eye(128, dtype=np.float32),
    }


CONST_SHAPES = {
    "wta": (128, 2, 3, 128), "wtb": (128, 2, 3, 128), "wsum": (128, 128),
    "wsc": (128, 4, 8), "we": (8, 4, 128), "wwf": (128, 4, 128),
    "bias": (128, 3), "idt8": (8, 8), "idt128": (128, 128),
}


# ----------------------------------------------------------------------------
# kernel program
# ----------------------------------------------------------------------------

def build_nc():
    import concourse.bass as bass
    import concourse.tile as tile
    import concourse.mybir as mybir
    from concourse import bacc
    from contextlib import ExitStack

    f32 = mybir.dt.float32
    f32r = mybir.dt.float32r
    ACT = mybir.ActivationFunctionType
    ALU = mybir.AluOpType
    AX = mybir.AxisListType

    nc = bacc.Bacc("TRN2", target_bir_lowering=False, debug=False)

    MM_CONSTS = {"wta", "wtb", "wsum", "wsc", "we", "wwf"}
    x = nc.dram_tensor("x", [B, 2, 128, T, 9, 130], f32r,
                       kind="ExternalInput").ap()
    cst = {n: nc.dram_tensor(n, list(s), f32r if n in MM_CONSTS else f32,
                             kind="ExternalInput").ap()
           for n, s in CONST_SHAPES.items()}
    out = nc.dram_tensor("out", [B, 128, HS, W], f32, kind="ExternalOutput").ap()

    def r(ap):
        return ap.bitcast(f32r)

    with tile.TileContext(nc) as tc, ExitStack() as ctx:
        cp = ctx.enter_context(tc.tile_pool(name="const", bufs=1))
        consts = {}
        for n, s in CONST_SHAPES.items():
            t = cp.tile(list(s), f32r if n in MM_CONSTS else f32, tag=n)
            nc.sync.dma_start(t[:], cst[n][:])
            consts[n] = t
        wta, wtb = consts["wta"], consts["wtb"]
        wsum, wsc, we, wwf = (consts["wsum"], consts["wsc"], consts["we"],
                              consts["wwf"])
        bias, idt8, idt128 = consts["bias"], consts["idt8"], consts["idt128"]

        xx_pool = ctx.enter_context(tc.tile_pool(name="xx", bufs=5))
        ref_pool = ctx.enter_context(tc.tile_pool(name="ref", bufs=4))
        emb_pool = ctx.enter_context(tc.tile_pool(name="emb", bufs=2))
        sml_pool = ctx.enter_context(tc.tile_pool(name="sml", bufs=2))
        prod_pool = ctx.enter_context(tc.tile_pool(name="prodp", bufs=4))
        out_pool = ctx.enter_context(tc.tile_pool(name="outb", bufs=2))
        ps1 = ctx.enter_context(tc.tile_pool(name="ps1", bufs=3, space="PSUM"))
        pss = ctx.enter_context(tc.tile_pool(name="pss", bufs=2, space="PSUM"))
        ps3 = ctx.enter_context(tc.tile_pool(name="ps3", bufs=3, space="PSUM"))

        state = {}

        def emit_a(u, inject):
            """Conv/scores phase for unit u; inject[t] () emitted after
            frame t's convs (PE-stream interleaving for unit u-1)."""
            b, s = divmod(u, 2)
            xxh = []
            for hf in range(2):
                xt = xx_pool.tile([128, 4, 9, 130], f32r, tag="xx",
                                  name=f"xx{u}_{hf}")
                nc.sync.dma_start(xt[:], x[b, s, :, 4 * hf:4 * (hf + 1)])
                xxh.append(xt)

            def xx(t):
                return xxh[t // 4][:, t % 4]

            sums = [pss.tile([128, 512], f32, tag="sum", name=f"sum{u}_{ch}")
                    for ch in range(NCH)]
            refs = []
            embs = []
            for t in range(T):
                j, sw = divmod(t, 2)
                if sw == 0:
                    refj = ref_pool.tile([128, NCH, 512], f32, tag="embref",
                                         name=f"ref{u}_{j}")
                    embj = emb_pool.tile([128, NCH, 512], f32r, tag="emb",
                                         name=f"emb{u}_{j}")
                    refs.append(refj)
                    embs.append(embj)
                refj, embj = refs[j], embs[j]
                cps = [ps1.tile([128, 512], f32, tag="cps",
                                name=f"cps{u}_{t}_{ch}")
                       for ch in range(NCH)]
                for ch in range(NCH):
                    r0 = 4 * ch
                    for dj in range(3):
                        nc.tensor.matmul(
                            cps[ch][:], wta[:, sw, dj, :],
                            xx(t)[:, r0:r0 + 4, dj:dj + 128],
                            start=(dj == 0), stop=False)
                for dj in range(3):
                    nc.tensor.matmul(
                        cps[0][:], wtb[0:64, sw, dj, :],
                        xx(t)[0:64, 2:6, dj:dj + 128],
                        start=False, stop=(dj == 2))
                    nc.tensor.matmul(
                        cps[1][:], wtb[64:128, sw, dj, :],
                        xx(t)[64:128, 5:9, dj:dj + 128],
                        start=False, stop=(dj == 2))
                for ch in range(NCH):
                    if sw == 0:
                        nc.scalar.activation(
                            refj[0:64, ch, :], cps[ch][0:64, :],
                            ACT.Identity, bias=bias[0:64, 0:1])
                        nc.scalar.activation(
                            embj[64:128, ch, :], cps[ch][64:128, :],
                            ACT.Copy)
                    else:
                        nc.scalar.activation(
                            embj[0:64, ch, :], cps[ch][0:64, :],
                            ACT.Copy)
                        nc.scalar.activation(
                            refj[64:128, ch, :], cps[ch][64:128, :],
                            ACT.Identity, bias=bias[64:128, 0:1])
                if sw == 1:
                    for ch in range(NCH):
                        nc.tensor.matmul(sums[ch][:], wsum[:],
                                         embj[:, ch, :],
                                         start=(j == 0), stop=(j == 3))
                fn = inject.get(t)
                if fn:
                    fn()

            esum = sml_pool.tile([128, NCH, 512], f32, tag="esum",
                                 name=f"esum{u}")
            for ch in range(NCH):
                nc.scalar.activation(esum[:, ch, :], sums[ch][:],
                                     ACT.Identity, bias=bias[:, 1:2])
            scps = [ps3.tile([8, 512], f32, tag="soft",
                             name=f"scps{u}_{ch}") for ch in range(NCH)]
            prods = []
            for j in range(4):
                prodj = prod_pool.tile([128, NCH, 512], f32r, tag="prod",
                                       name=f"prod{u}_{j}")
                nc.vector.tensor_mul(prodj[:].opt(), refs[j][:].opt(),
                                     esum[:].opt())
                prods.append(prodj)
            for j in range(4):
                for ch in range(NCH):
                    nc.tensor.matmul(scps[ch][:], wsc[:, j, :],
                                     prods[j][:, ch, :],
                                     start=(j == 0), stop=(j == 3))
            state[u] = {"xxh": xxh, "scps": scps}

        def b_pieces(u):
            """B-phase for unit u as injectable pieces keyed by slot."""
            b, s = divmod(u, 2)
            st = state[u]

            def xx(t):
                return st["xxh"][t // 4][:, t % 4]

            def p_trans():
                scs = sml_pool.tile([8, NCH, 512], f32, tag="scs",
                                    name=f"scs{u}")
                for ch in range(NCH):
                    nc.scalar.activation(scs[:, ch, :], st["scps"][ch][:],
                                         ACT.Identity, scale=2.0)
                psT = pss.tile([128, HS, 8], f32, tag="sum", name=f"psT{u}")
                for h in range(HS):
                    nc.tensor.transpose(
                        psT[:, h, :],
                        scs[:].opt()[:, 128 * h:128 * (h + 1)], idt8[:])
                st["psT"] = psT

            def p_soft():
                psT = st["psT"]
                nm = sml_pool.tile([128, HS], f32, tag="nm", name=f"nm{u}")
                nc.vector.tensor_reduce(nm[:], psT[:], axis=AX.X,
                                        op=ALU.max, negate=True)
                et = sml_pool.tile([128, HS, 8], f32, tag="et",
                                   name=f"et{u}")
                for h in range(HS):
                    nc.scalar.activation(et[:, h, :], psT[:, h, :], ACT.Exp,
                                         bias=nm[:, h:h + 1])
                den = sml_pool.tile([128, HS], f32, tag="den",
                                    name=f"den{u}")
                nc.vector.tensor_reduce(den[:], et[:], axis=AX.X, op=ALU.add)
                rec = sml_pool.tile([128, HS], f32, tag="rec",
                                    name=f"rec{u}")
                nc.vector.reciprocal(rec[:], den[:])
                attnT = sml_pool.tile([128, HS, 8], f32, tag="attnT",
                                      name=f"attnT{u}")
                for h in range(HS):
                    nc.vector.tensor_scalar_mul(attnT[:, h, :], et[:, h, :],
                                                rec[:, h:h + 1])
                st["attnT"] = attnT

            def p_btrans():
                attnT = st["attnT"]
                attn_ps = [ps3.tile([8, 512], f32, tag="soft",
                                    name=f"attnps{u}_{ch}")
                           for ch in range(NCH)]
                for h in range(HS):
                    nc.tensor.transpose(
                        attn_ps[h // 4][:, 128 * (h % 4):128 * (h % 4 + 1)],
                        attnT[:, h, :], idt128[:])
                attn = sml_pool.tile([8, NCH, 512], f32r, tag="attn",
                                     name=f"attn{u}")
                for ch in range(NCH):
                    nc.scalar.activation(attn[:, ch, :], attn_ps[ch][:],
                                         ACT.Copy)
                st["attn"] = attn

            def p_wj(j):
                def fn():
                    attn = st["attn"]
                    wts = st.setdefault("wts", [])
                    for ch in range(NCH):
                        abc = ps3.tile([128, 4, 128], f32, tag="soft",
                                       name=f"abc{u}_{j}_{ch}")
                        nc.tensor.matmul(abc[:].opt(), we[:, j, :],
                                         attn[:, ch, :],
                                         start=True, stop=True)
                        wt = sml_pool.tile([128, 4, 128], f32r, tag="wt",
                                           bufs=5, name=f"wt{u}_{j}_{ch}")
                        nc.vector.tensor_mul(
                            wt[0:64],
                            xx(2 * j)[0:64, 1 + 4 * ch:5 + 4 * ch, 1:129]
                            .bitcast(f32), abc[0:64])
                        nc.vector.tensor_mul(
                            wt[64:128],
                            xx(2 * j + 1)[64:128, 4 * ch:4 + 4 * ch, 1:129]
                            .bitcast(f32), abc[64:128])
                        wts.append(wt)
                return fn

            def p_wf():
                wts = st["wts"]
                ops = [ps1.tile([128, 512], f32, tag="cps",
                                name=f"ops{u}_{ch}") for ch in range(NCH)]
                for j in range(4):
                    for ch in range(NCH):
                        nc.tensor.matmul(ops[ch][:], wwf[:, j, :],
                                         wts[2 * j + ch][:].opt(),
                                         start=(j == 0), stop=(j == 3))
                outbuf = state.get(("ob", b))
                if outbuf is None:
                    outbuf = out_pool.tile([128, HS, W], f32, tag="outbuf",
                                           name=f"outbuf{b}")
                    state[("ob", b)] = outbuf
                for ch in range(NCH):
                    yb = sml_pool.tile([128, 512], f32, tag="yb",
                                       name=f"yb{u}_{ch}")
                    nc.scalar.activation(
                        yb[64 * s:64 * (s + 1), :],
                        ops[ch][64 * s:64 * (s + 1), :],
                        ACT.Identity, bias=bias[64 * s:64 * (s + 1), 2:3])
                    nc.vector.scalar_tensor_tensor(
                        outbuf[64 * s:64 * (s + 1),
                               4 * ch:4 * (ch + 1), :].opt(),
                        yb[64 * s:64 * (s + 1), :], 0.1,
                        yb[64 * s:64 * (s + 1), :],
                        op0=ALU.mult, op1=ALU.max)
                if s == 1:
                    nc.sync.dma_start(out[b], outbuf[:])
                    del state[("ob", b)]
                del state[u]

            return {0: p_trans, 1: p_soft, 2: p_btrans,
                    3: p_wj(0), 4: p_wj(1), 5: p_wj(2), 6: p_wj(3),
                    7: p_wf}

        emit_a(0, {})
        for u in range(1, 8):
            emit_a(u, b_pieces(u - 1))
        # drain last unit standalone
        tail = b_pieces(7)
        for k in range(8):
            tail[k]()

    nc.compile()
    return nc


# ----------------------------------------------------------------------------
# entry point
# ----------------------------------------------------------------------------

def kernel(aligned_feat, w1, b1, w2, b2, wf, bf):
    from concourse import bass_utils

    if "nc" not in _CACHE:
        _CACHE["nc"] = build_nc()
    nc = _CACHE["nc"]

    A = _stage_inputs(aligned_feat)
    consts = _make_consts(w1, b1, w2, b2, wf, bf)
    in_maps = [{"x": A[k], **consts} for k in range(NCORES)]

    res = bass_utils.run_bass_kernel_spmd(nc, in_maps, core_ids=list(range(NCORES)))
    outs = [res.results[k]["out"] for k in range(NCORES)]  # [B,128,HS,W]

    full = np.empty((B, C, H, W), np.float32)
    for k in range(NCORES):
        o = outs[k]
        for s in range(2):
            full[:, :, RPC * k + HS * s: RPC * k + HS * (s + 1), :] = \
                o[:, 64 * s:64 * (s + 1), :, :]
    return full
